# revision 21
# baseline (speedup 1.0000x reference)
"""Trainium2 Bass kernel for nn_NetworkAdditive (LSTM-gain network).

Computation (see the reference):
  x2n[t,b,n]  = input[t,b,:] @ W2n[n,:] + b2n[n]
  per-step LSTM cell over state [B,N,H] with scalar input x2n[t,b,n]:
     gates = x*w_ih + h @ W_hh^T + (b_ih+b_hh); i,f,g,o split
     c = sig(f)*c + sig(i)*tanh(g);  h = sig(o)*tanh(c)
     g_t[b,n] = h @ Wg[0] + bg
  gain[t] = g_0 (t=0), 1 (t=1), g_{t-2} (t>=2)
  out_n[t,b,n] = gain * softplus(x2n - 1)
  out[t,b,p] = out_n @ Wout[p,:] + bout

Sharding: data-parallel over batch B across 8 cores (8 batches/core).
Within a core the 2048 state rows (b,n) are SBUF-resident in a transposed
"block diagonal" layout: tiles [128, 1024] whose partitions 0-63 hold h^T
for rows 0-1023 and partitions 64-127 hold h^T for rows 1024-2047.  Gate
matmuls use K=128 block-diagonal weights; the x-term enters as a rank-2
PSUM-accumulated matmul; biases ride ScalarE's per-partition activation
bias.  g_t is accumulated into one PSUM bank over 32-step windows via a
sliding slice of a constant weight matrix.

Written in *raw* Bass (explicit engine blocks + semaphores): the walrus
build in this container only supports ONE sync-wait per instruction, so
Tile's auto-generated multi-wait sync_info cannot be compiled.  All waits
are standalone wait_ge instructions.
"""

import os
import sys
import numpy as np

for _p in ("/opt/trn_rl_repo",):
    if _p not in sys.path and os.path.isdir(_p):
        sys.path.insert(0, _p)

T, B, N, H, D, P = 512, 64, 256, 64, 128, 64
USE_F32R = os.environ.get("K_F32R", "0") == "1"
NCORES = 8
BC = B // NCORES           # batches per core = 8
GBASE = (0, 64, 128, 192)  # torch LSTM gate order: i, f, g(cell), o
F32 = np.float32


# ----------------------------------------------------------------------------
# Host-side weight preparation (shared across all cores)
# ----------------------------------------------------------------------------
def _prep_shared(W2n, b2n, W_ih, W_hh, b_ih, b_hh, Wg, bg, Wout, bout):
    S = {}
    bias = (np.asarray(b_ih, F32) + np.asarray(b_hh, F32))
    W_hh = np.asarray(W_hh, F32)
    W_ih = np.asarray(W_ih, F32)
    wbd = np.zeros((128, 4, 128), F32)   # block-diag lhsT per gate [k, X, m]
    xw = np.zeros((2, 4, 128), F32)      # x-inject lhsT per gate  [k, X, m]
    gb = np.zeros((128, 4), F32)         # ACT bias per gate       [m, X]
    for X in range(4):
        b0 = GBASE[X]
        Wx = W_hh[b0:b0 + 64, :]         # [unit, h]
        wbd[0:64, X, 0:64] = Wx.T
        wbd[64:128, X, 64:128] = Wx.T
        wx = W_ih[b0:b0 + 64, 0]
        xw[0, X, 0:64] = wx
        xw[1, X, 64:128] = wx
        gb[0:64, X] = bias[b0:b0 + 64]
        gb[64:128, X] = bias[b0:b0 + 64]
    big = np.zeros((128, 253), F32)      # sliding g-readout weights
    big[0:64, 125] = np.asarray(Wg, F32)[0, :]
    big[64:128, 127] = np.asarray(Wg, F32)[0, :]

    S["wbd"] = wbd
    S["xw"] = xw
    S["gb"] = gb
    S["big"] = big
    S["w2nt"] = np.ascontiguousarray(np.asarray(W2n, F32).T)       # [128, 256]
    spb = np.empty((128, 2), F32)
    spb[:, 0] = np.asarray(b2n, F32)[0:128] - 1.0
    spb[:, 1] = np.asarray(b2n, F32)[128:256] - 1.0
    S["spbias"] = spb
    S["b2nrow"] = np.asarray(b2n, F32).reshape(1, N).copy()
    S["ones1"] = np.ones((1, 128), F32)
    S["ident"] = np.eye(128, dtype=F32)
    wo = np.empty((128, 2, P), F32)
    Wout = np.asarray(Wout, F32)
    wo[:, 0, :] = Wout[:, 0:128].T
    wo[:, 1, :] = Wout[:, 128:256].T
    S["woutt"] = wo
    S["boutrow"] = np.asarray(bout, F32).reshape(1, P).copy()
    S["bg0"] = float(np.asarray(bg, F32)[0])
    return S


# ----------------------------------------------------------------------------
# Numpy mirror of the device program (validates the layout algebra)
# ----------------------------------------------------------------------------
def _sigmoid(x):
    return 1.0 / (1.0 + np.exp(-x))


def _mirror_core(inp_c, S, TT):
    w2nt = S["w2nt"]
    inputT = inp_c.T                                       # [128 d, TT*8]
    x2n_rows = inputT.T @ w2nt + S["b2nrow"]               # [TT*8, 256]
    xdram = x2n_rows.reshape(TT, 2 * 1024)
    spT = np.empty((2, 128, TT * 8), F32)
    for c in range(2):
        ps = w2nt[:, c * 128:(c + 1) * 128].T @ inputT
        z = ps + S["spbias"][:, c:c + 1]
        spT[c] = np.log1p(np.exp(z))
    h2 = np.zeros((128, 1024), F32)
    c2 = np.zeros((128, 1024), F32)
    nwin = TT // 32
    g_sb = np.zeros((128, nwin * 512), F32)
    for d in range(nwin):
        gps = np.zeros((128, 512), F32)
        for k in range(32):
            t = d * 32 + k
            xs = xdram[t].reshape(2, 1024)
            act = []
            for X in range(4):
                ps = np.empty((128, 1024), F32)
                for half in range(2):
                    sl = slice(half * 512, (half + 1) * 512)
                    ps[:, sl] = (S["wbd"][:, X, :].T @ h2[:, sl]
                                 + S["xw"][:, X, :].T @ xs[:, sl])
                act.append(ps + S["gb"][:, X:X + 1])
            sigi, sigf = _sigmoid(act[0]), _sigmoid(act[1])
            tang, sigo = np.tanh(act[2]), _sigmoid(act[3])
            c2 = c2 * sigf + sigi * tang
            h2 = sigo * np.tanh(c2)
            gps += S["big"][:, 125 - 4 * k:253 - 4 * k].T @ h2[:, 0:512]
            gps += S["big"][:, 124 - 4 * k:252 - 4 * k].T @ h2[:, 512:1024]
        g_sb[:, d * 512:(d + 1) * 512] = gps
    # phase C: block transpose of g_sb, then gain multiply (in place on spT)
    gT = np.empty_like(g_sb)
    for dd in range(nwin):
        for j in range(4):
            off = dd * 512 + j * 128
            gT[:, off:off + 128] = g_sb[:, off:off + 128].T
    gT = gT + S["bg0"]
    view = gT.reshape(128, nwin, 4, 32, 4)                 # [p, d, j, k, q]
    onT = np.empty((2, 128, TT * 8), F32)
    for c in range(2):
        gv = view[:, :, c::2, :, :]                        # [p, d, j2, k, q]
        gv = gv.transpose(0, 1, 3, 4, 2).reshape(128, TT, 8)
        sp = spT[c].reshape(128, TT, 8)
        on = np.empty((128, TT, 8), F32)
        on[:, 0] = gv[:, 0] * sp[:, 0]
        on[:, 1] = sp[:, 1]
        on[:, 2:] = gv[:, 0:TT - 2] * sp[:, 2:]
        onT[c] = on.reshape(128, TT * 8)
    out = np.empty((TT * 8, P), F32)
    for m in range(TT * 8 // 128):
        sl = slice(m * 128, (m + 1) * 128)
        acc = (onT[0][:, sl].T @ S["woutt"][:, 0, :]
               + onT[1][:, sl].T @ S["woutt"][:, 1, :] + S["boutrow"])
        out[sl] = acc
    return out


def mirror(inputs, TT=T):
    S = _prep_shared(**{k: v for k, v in inputs.items() if k != "input"})
    inp = np.asarray(inputs["input"], F32)[:TT]
    outs = []
    for cidx in range(NCORES):
        inp_c = np.ascontiguousarray(
            inp[:, cidx * BC:(cidx + 1) * BC, :]).reshape(TT * BC, D)
        outs.append(_mirror_core(inp_c, S, TT).reshape(TT, BC, P))
    return np.concatenate(outs, axis=1)


# ----------------------------------------------------------------------------
# Raw Bass program
# ----------------------------------------------------------------------------
def _build_nc_with_bg(TT, bg0):
    from contextlib import ExitStack
    import concourse.bass as bass
    from concourse import mybir

    f32 = mybir.dt.float32
    AF = mybir.ActivationFunctionType
    nc = bass.Bass("TRN2", target_bir_lowering=False, debug=False,
                   num_devices=NCORES)

    nwin = TT // 32
    ntb = TT * BC // 128
    CB = TT * BC                       # columns per c-chunk in spT
    NCONST = 11

    inp = nc.dram_tensor("inp", [TT * BC, D], f32, kind="ExternalInput").ap()
    dio = {}
    for nm, shape in (("wbd", [128, 4, 128]), ("xw", [2, 4, 128]),
                      ("gb", [128, 4]), ("big", [128, 253]),
                      ("w2nt", [D, N]), ("spbias", [128, 2]),
                      ("b2nrow", [1, N]), ("ones1", [1, 128]),
                      ("ident", [128, 128]), ("woutt", [128, 2, P]),
                      ("boutrow", [1, P])):
        dio[nm] = nc.dram_tensor(nm, shape, f32, kind="ExternalInput").ap()
    xdram = nc.dram_tensor("xdram", [TT * BC, N], f32).ap()
    out = nc.dram_tensor("out", [TT * BC, P], f32, kind="ExternalOutput").ap()
    xdr = xdram.rearrange("(t e) n -> t (e n)", e=BC)      # [TT, 2048]

    with ExitStack() as ctx:
        E = ctx.enter_context
        # ---------------- SBUF
        wbd = E(nc.sbuf_tensor([128, 4, 128], f32))
        xw = E(nc.sbuf_tensor([2, 4, 128], f32))
        gbb = E(nc.sbuf_tensor([128, 4], f32))
        big = E(nc.sbuf_tensor([128, 253], f32))
        w2nt = E(nc.sbuf_tensor([128, N], f32))
        spbias = E(nc.sbuf_tensor([128, 2], f32))
        b2nrow = E(nc.sbuf_tensor([1, N], f32))
        ones1 = E(nc.sbuf_tensor([1, 128], f32))
        ident = E(nc.sbuf_tensor([128, 128], f32))
        woutt = E(nc.sbuf_tensor([128, 2, P], f32))
        boutrow = E(nc.sbuf_tensor([1, P], f32))
        consts = [("wbd", wbd), ("xw", xw), ("gb", gbb), ("big", big),
                  ("w2nt", w2nt), ("spbias", spbias), ("b2nrow", b2nrow),
                  ("ones1", ones1), ("ident", ident), ("woutt", woutt),
                  ("boutrow", boutrow)]
        assert len(consts) == NCONST

        spT = E(nc.sbuf_tensor([128, 2 * TT * BC], f32))   # softplus -> onT
        g_sb = E(nc.sbuf_tensor([128, nwin * 512], f32))   # g -> gT in place
        inputT = E(nc.sbuf_tensor([128, TT * BC], f32))
        h2T = E(nc.sbuf_tensor([128, 1024], f32))
        c2 = E(nc.sbuf_tensor([128, 1024], f32))
        sigi = E(nc.sbuf_tensor([128, 2, 1024], f32))      # parity-buffered
        sigf = E(nc.sbuf_tensor([128, 2, 1024], f32))
        tang = E(nc.sbuf_tensor([128, 2, 1024], f32))
        sigo = E(nc.sbuf_tensor([128, 2, 1024], f32))
        tch = E(nc.sbuf_tensor([128, 2, 512], f32))        # [A|B] halves
        tm = E(nc.sbuf_tensor([128, 512], f32))
        xsr = E(nc.sbuf_tensor([2, 4, 1024], f32))         # xs ring of 4
        tin = E(nc.sbuf_tensor([128, 3, 128], f32))
        xr = E(nc.sbuf_tensor([128, 3, N], f32))
        et = E(nc.sbuf_tensor([128, 2, 512], f32))
        ot = E(nc.sbuf_tensor([128, 3, P], f32))

        # ---------------- semaphores
        sems = {}
        for snm in ("cdma", "peA", "dveA", "peB", "dveB",
                    "peC", "actsp", "ms", "pe", "act", "dve", "gd",
                    "peD", "dveC", "dveD", "peE", "dveE"):
            sems[snm] = E(nc.semaphore(f"s_{snm}"))
        # per-ring-slot DMA sems: HWDGE completions are NOT ordered across
        # transfers, so a shared counting sem cannot be waited on mid-stream;
        # one sem per ring slot keeps a single outstanding DMA per sem.
        sems["indma"] = [E(nc.semaphore(f"s_indma{i}")) for i in range(3)]
        sems["xs"] = [E(nc.semaphore(f"s_xs{i}")) for i in range(4)]
        sems["od"] = [E(nc.semaphore(f"s_od{i}")) for i in range(3)]
        sems["xwd"] = [E(nc.semaphore(f"s_xwd{i}")) for i in range(3)]
        S = type("Sems", (), sems)

        # ---------------- PSUM regions (aliased across phases; the phases
        # are temporally separated by explicit cross-engine waits)
        pa = ExitStack()
        pt = pa.enter_context(nc.psum_tensor([128, 2, 512], f32))
        psxr = pa.enter_context(nc.psum_tensor([128, 2, 512], f32))
        pssp = pa.enter_context(nc.psum_tensor([128, 2, 512], f32))
        pa.close()
        pb = ExitStack()
        psi = pb.enter_context(nc.psum_tensor([128, 1024], f32))
        psf = pb.enter_context(nc.psum_tensor([128, 1024], f32))
        psc = pb.enter_context(nc.psum_tensor([128, 1024], f32))
        pso = pb.enter_context(nc.psum_tensor([128, 512], f32))
        gps = pb.enter_context(nc.psum_tensor([128, 512], f32))
        pb.close()
        pc = ExitStack()
        ptc = pc.enter_context(nc.psum_tensor([128, 2, 512], f32))
        pproj = pc.enter_context(nc.psum_tensor([128, 4, 512], f32))
        pc.close()

        nspit = 2 * TT * BC // 512         # A3 iterations

        if USE_F32R:
            rr = lambda ap: ap.bitcast(mybir.dt.float32r)
        else:
            rr = lambda ap: ap

        # number of DVE ops in phase C2 (gates phase C3)
        D2TOT = 1 + 2 * (1 + sum(1 for dp in range(nwin)
                                 if min(32, TT - 2 - dp * 32) > 0))

        def W(eng, sem, v):
            if v > 0:
                eng.wait_ge(sem, v)

        with nc.Block() as block:
            # ================= SP: all DMA =================
            @block.sync
            def _(sp):
                for nm, hdl in consts:
                    sp.dma_start(out=hdl[:], in_=dio[nm]).then_inc(S.cdma, 16)
                # A1: input tiles in
                for ch in range(ntb):
                    W(sp, S.peA, ch - 2)
                    sp.dma_start(
                        out=tin[:, ch % 3, :],
                        in_=inp[ch * 128:(ch + 1) * 128, :],
                    ).then_inc(S.indma[ch % 3], 16)
                # A2: x2n rows out to DRAM
                for ch in range(ntb):
                    W(sp, S.dveB, ch + 1)
                    sp.dma_start(
                        out=xdram[ch * 128:(ch + 1) * 128, :],
                        in_=xr[:, ch % 3, :],
                    ).then_inc(S.xwd[ch % 3], 16)
                # B: xs stream (SP HWDGE is FIFO, so these start only after
                # the A2 writes above have drained)
                for t in range(TT):
                    W(sp, S.pe, 6 * (t - 3))
                    sp.dma_start(
                        out=xsr[:, t % 4, :],
                        in_=xdr[t].rearrange("(a b) -> a b", a=2),
                    ).then_inc(S.xs[t % 4], 16)
                # C3: output
                for m in range(ntb):
                    W(sp, S.dveE, m + 1)
                    sp.dma_start(
                        out=out[m * 128:(m + 1) * 128, :],
                        in_=ot[:, m % 3, :],
                    ).then_inc(S.od[m % 3], 16)

            # ================= PE =================
            @block.tensor
            def _(pe):
                MM = nc.tensor.matmul
                # A1 transposes
                pe.wait_ge(S.cdma, 16 * NCONST)
                for ch in range(ntb):
                    pe.wait_ge(S.indma[ch % 3], 16 * (ch // 3 + 1))
                    W(pe, S.dveA, ch - 1)
                    nc.tensor.transpose(
                        pt[:, ch % 2, 0:128], tin[:, ch % 3, :], ident[:]
                    ).then_inc(S.peA, 1)
                # A2 x2n rows
                pe.wait_ge(S.dveA, ntb)
                for ch in range(ntb):
                    W(pe, S.dveB, ch - 1)
                    MM(psxr[:, ch % 2, 0:N],
                       inputT[:, ch * 128:(ch + 1) * 128], w2nt[:],
                       start=True, stop=False)
                    MM(psxr[:, ch % 2, 0:N], ones1[:], b2nrow[:],
                       start=False, stop=True).then_inc(S.peB, 1)
                # A3 softplus matmuls (transposed x2n)
                for it in range(nspit):
                    c, ch = divmod(it, TT * BC // 512)
                    W(pe, S.actsp, 2 * (it - 1))
                    MM(pssp[:, it % 2, :], w2nt[:, c * 128:(c + 1) * 128],
                       inputT[:, ch * 512:(ch + 1) * 512],
                       start=True, stop=True).then_inc(S.peC, 1)
                # B: the scan
                pe.wait_ge(S.actsp, 2 * nspit)
                pe.wait_ge(S.dveB, ntb)
                pe.wait_ge(S.ms, 2)
                for t in range(TT):
                    k, d = t % 32, t // 32
                    pe.wait_ge(S.xs[t % 4], 16 * (t // 4 + 1))
                    W(pe, S.dve, 8 * t)
                    xs_t = xsr[:, t % 4, :]
                    for X, ps, rel in ((0, psi, 1), (1, psf, 2), (2, psc, 3)):
                        W(pe, S.act, 7 * (t - 1) + rel)
                        for half in range(2):
                            sl = slice(half * 512, (half + 1) * 512)
                            MM(ps[:, sl], rr(wbd[:, X, :]), rr(h2T[:, sl]),
                               start=True, stop=False)
                            mm2 = MM(ps[:, sl], rr(xw[:, X, :]),
                                     rr(xs_t[:, sl]), start=False, stop=True)
                        mm2.then_inc(S.pe, 1)
                    W(pe, S.act, 7 * (t - 1) + 5)
                    MM(pso[:], rr(wbd[:, 3, :]), rr(h2T[:, 0:512]),
                       start=True, stop=False)
                    MM(pso[:], rr(xw[:, 3, :]), rr(xs_t[:, 0:512]),
                       start=False, stop=True).then_inc(S.pe, 1)
                    W(pe, S.act, 7 * t + 4)
                    MM(pso[:], rr(wbd[:, 3, :]), rr(h2T[:, 512:1024]),
                       start=True, stop=False)
                    MM(pso[:], rr(xw[:, 3, :]), rr(xs_t[:, 512:1024]),
                       start=False, stop=True).then_inc(S.pe, 1)
                    pe.wait_ge(S.dve, 8 * t + 8)
                    if k == 0:
                        W(pe, S.gd, d)
                    MM(gps[:], rr(big[:, 125 - 4 * k:253 - 4 * k]),
                       rr(h2T[:, 0:512]), start=(k == 0), stop=False,
                       skip_group_check=True)
                    MM(gps[:], rr(big[:, 124 - 4 * k:252 - 4 * k]),
                       rr(h2T[:, 512:1024]), start=False, stop=(k == 31),
                       skip_group_check=True).then_inc(S.pe, 1)
                # C1 block transposes of g_sb
                pe.wait_ge(S.act, 7 * TT)
                pe.wait_ge(S.gd, nwin)
                for dd in range(nwin):
                    W(pe, S.dveC, dd - 1)
                    for j in range(4):
                        mmt = nc.tensor.transpose(
                            ptc[:, dd % 2, j * 128:(j + 1) * 128],
                            g_sb[:, dd * 512 + j * 128:
                                 dd * 512 + (j + 1) * 128],
                            ident[:])
                    mmt.then_inc(S.peD, 1)
                # C3 projection
                pe.wait_ge(S.dveD, D2TOT)
                for m in range(ntb):
                    W(pe, S.dveE, m - 3)
                    MM(pproj[:, m % 4, 0:P], spT[:, m * 128:(m + 1) * 128],
                       woutt[:, 0, :], start=True, stop=False)
                    MM(pproj[:, m % 4, 0:P],
                       spT[:, CB + m * 128: CB + (m + 1) * 128],
                       woutt[:, 1, :], start=False, stop=False)
                    MM(pproj[:, m % 4, 0:P], ones1[:], boutrow[:],
                       start=False, stop=True).then_inc(S.peE, 1)

            # ================= ACT =================
            @block.scalar
            def _(act):
                ACT = nc.scalar.activation
                # A3 softplus: sp = ln(1 + exp(x2n + b2n - 1))
                for it in range(nspit):
                    c, ch = divmod(it, TT * BC // 512)
                    act.wait_ge(S.peC, max(it + 1, 2))
                    W(act, S.actsp, 2 * it - 2)
                    ACT(out=et[:, it % 2, :], in_=pssp[:, it % 2, :],
                        func=AF.Exp, bias=spbias[:, c:c + 1],
                        scale=1.0).then_inc(S.actsp, 1)
                    act.wait_ge(S.actsp, 2 * it + 1)
                    ACT(out=spT[:, c * CB + ch * 512:
                                c * CB + (ch + 1) * 512],
                        in_=et[:, it % 2, :], func=AF.Ln, bias=1.0,
                        scale=1.0).then_inc(S.actsp, 1)
                # B: gates
                for t in range(TT):
                    par = t % 2
                    W(act, S.dve, 8 * t)
                    act.wait_ge(S.pe, max(6 * t + 1, 4))
                    ACT(out=sigi[:, par, :], in_=psi[:], func=AF.Sigmoid,
                        bias=gbb[:, 0:1], scale=1.0).then_inc(S.act, 1)
                    act.wait_ge(S.pe, max(6 * t + 2, 4))
                    ACT(out=sigf[:, par, :], in_=psf[:], func=AF.Sigmoid,
                        bias=gbb[:, 1:2], scale=1.0).then_inc(S.act, 1)
                    act.wait_ge(S.pe, max(6 * t + 3, 4))
                    ACT(out=tang[:, par, :], in_=psc[:], func=AF.Tanh,
                        bias=gbb[:, 2:3], scale=1.0).then_inc(S.act, 1)
                    act.wait_ge(S.pe, max(6 * t + 4, 4))
                    ACT(out=sigo[:, par, 0:512], in_=pso[:], func=AF.Sigmoid,
                        bias=gbb[:, 3:4], scale=1.0).then_inc(S.act, 1)
                    act.wait_ge(S.pe, 6 * t + 5)
                    ACT(out=sigo[:, par, 512:1024], in_=pso[:],
                        func=AF.Sigmoid, bias=gbb[:, 3:4],
                        scale=1.0).then_inc(S.act, 1)
                    act.wait_ge(S.dve, 8 * t + 3)
                    ACT(out=tch[:, 0, :], in_=c2[:, 0:512],
                        func=AF.Tanh).then_inc(S.act, 1)
                    act.wait_ge(S.dve, 8 * t + 6)
                    ACT(out=tch[:, 1, :], in_=c2[:, 512:1024],
                        func=AF.Tanh).then_inc(S.act, 1)

            # ================= DVE =================
            @block.vector
            def _(dve):
                mul = nc.vector.tensor_mul
                add = nc.vector.tensor_add
                cpy = nc.vector.tensor_copy
                # A1 copies
                for ch in range(ntb):
                    dve.wait_ge(S.peA, ch + 1)
                    cpy(inputT[:, ch * 128:(ch + 1) * 128],
                        pt[:, ch % 2, 0:128]).then_inc(S.dveA, 1)
                # A2 copies
                for ch in range(ntb):
                    dve.wait_ge(S.peB, max(ch + 1, 2))
                    if ch >= 3:
                        dve.wait_ge(S.xwd[ch % 3], 16 * ((ch - 3) // 3 + 1))
                    cpy(xr[:, ch % 3, :],
                        psxr[:, ch % 2, 0:N]).then_inc(S.dveB, 1)
                # scan state init
                nc.vector.memset(h2T[:], 0.0).then_inc(S.ms, 1)
                nc.vector.memset(c2[:], 0.0).then_inc(S.ms, 1)
                # B: state update
                for t in range(TT):
                    par = t % 2
                    k = t % 32
                    b8 = 8 * t
                    A, Bh = slice(0, 512), slice(512, 1024)
                    # every op incs s_dve and waits on the previous value:
                    # dependent same-engine ops must be sem-ordered (the DVE
                    # pipeline drains per op on HW; the sim requires the sem)
                    dve.wait_ge(S.act, 7 * t + 2)
                    mul(c2[:, A], c2[:, A], sigf[:, par, A]).then_inc(S.dve, 1)
                    dve.wait_ge(S.act, 7 * t + 3)
                    mul(tm[:], sigi[:, par, A],
                        tang[:, par, A]).then_inc(S.dve, 1)
                    dve.wait_ge(S.dve, b8 + 2)
                    add(c2[:, A], c2[:, A], tm[:]).then_inc(S.dve, 1)
                    mul(c2[:, Bh], c2[:, Bh],
                        sigf[:, par, Bh]).then_inc(S.dve, 1)
                    dve.wait_ge(S.dve, b8 + 3)
                    mul(tm[:], sigi[:, par, Bh],
                        tang[:, par, Bh]).then_inc(S.dve, 1)
                    dve.wait_ge(S.dve, b8 + 5)
                    add(c2[:, Bh], c2[:, Bh], tm[:]).then_inc(S.dve, 1)
                    dve.wait_ge(S.act, 7 * t + 6)
                    mul(h2T[:, A], sigo[:, par, A],
                        tch[:, 0, :]).then_inc(S.dve, 1)
                    dve.wait_ge(S.act, 7 * t + 7)
                    mul(h2T[:, Bh], sigo[:, par, Bh],
                        tch[:, 1, :]).then_inc(S.dve, 1)
                    if k == 31:
                        d = t // 32
                        dve.wait_ge(S.pe, 6 * t + 6)
                        cpy(g_sb[:, d * 512:(d + 1) * 512],
                            gps[:]).then_inc(S.gd, 1)
                # C1: write transposed blocks back into g_sb
                for dd in range(nwin):
                    dve.wait_ge(S.peD, max(dd + 1, 2))
                    cpy(g_sb[:, dd * 512:(dd + 1) * 512],
                        ptc[:, dd % 2, :]).then_inc(S.dveC, 1)
                # C2: gain multiply, in place into spT
                dve.wait_ge(S.dveC, nwin)
                nc.vector.tensor_scalar_add(g_sb[:], g_sb[:],
                                            bg0).then_inc(S.dveD, 1)
                gview = g_sb[:].rearrange(
                    "p (dd j2 jc k q) -> p dd j2 jc k q",
                    dd=nwin, j2=2, jc=2, k=32, q=4)
                for c in range(2):
                    base = c * CB
                    g0 = gview[:, 0, :, c, 0, :].transpose([0, 2, 1])
                    sp0 = spT[:, base:base + 8].rearrange(
                        "p (q l) -> p q l", q=4)
                    dve.wait_ge(S.dveD, 1)
                    mul(sp0, g0, sp0).then_inc(S.dveD, 1)
                    for dp in range(nwin):
                        kk = min(32, TT - 2 - dp * 32)
                        if kk <= 0:
                            break
                        src = gview[:, dp, :, c, 0:kk, :].transpose(
                            [0, 2, 3, 1])
                        t0 = dp * 32 + 2
                        dst = spT[:, base + t0 * 8: base + (t0 + kk) * 8] \
                            .rearrange("p (k q l) -> p k q l", q=4, l=2)
                        dve.wait_ge(S.dveD, 1)
                        mul(dst, src, dst).then_inc(S.dveD, 1)
                # C3 copies
                for m in range(ntb):
                    dve.wait_ge(S.peE, max(m + 1, 4))
                    if m >= 3:
                        dve.wait_ge(S.od[m % 3], 16 * ((m - 3) // 3 + 1))
                    cpy(ot[:, m % 3, :],
                        pproj[:, m % 4, 0:P]).then_inc(S.dveE, 1)

    return nc


# ----------------------------------------------------------------------------
# Entry point
# ----------------------------------------------------------------------------
_CACHE = {}
_last_results = None


def kernel(input, W2n, b2n, W_ih, W_hh, b_ih, b_hh, Wg, bg, Wout, bout,
           _TT=None, _trace=False):
    global _last_results
    from concourse.bass_utils import run_bass_kernel_spmd

    TT = _TT or T
    S = _prep_shared(W2n, b2n, W_ih, W_hh, b_ih, b_hh, Wg, bg, Wout, bout)
    if TT not in _CACHE:
        _CACHE[TT] = _build_nc_with_bg(TT, S["bg0"])
    nc = _CACHE[TT]

    inp = np.asarray(input, F32)[:TT]
    shared = {k: S[k] for k in ("wbd", "xw", "gb", "big", "w2nt", "spbias",
                                "b2nrow", "ones1", "ident", "woutt",
                                "boutrow")}
    in_maps = []
    for cidx in range(NCORES):
        inp_c = np.ascontiguousarray(
            inp[:, cidx * BC:(cidx + 1) * BC, :]).reshape(TT * BC, D)
        m = dict(shared)
        m["inp"] = inp_c
        in_maps.append(m)

    res = run_bass_kernel_spmd(nc, in_maps, list(range(NCORES)), trace=_trace)
    _last_results = res
    outs = [res.results[c]["out"].reshape(TT, BC, P) for c in range(NCORES)]
    return np.concatenate(outs, axis=1)
